# revision 64
# baseline (speedup 1.0000x reference)
"""CrossViewEnhancement Trainium2 kernel (8-core batch-parallel SPMD).

Reference computation (per batch element):
    q = avgpool2(conv1x1(bev_x, qw, qb))                   [C8, 64, 64]
    k = avgpool2(conv1x1(front_x, kw, kb)).mean(h)         [C8, 64]
    v = avgpool2(conv3x3(front_x, vw, vb, pad=1))          [C, 64, 64]
    e = einsum(k, q); L2-normalize over h per column       [64, 64]
    T = e * v.sum(h); nn-upsample x2                       [C, 128, 128]
    out = cat(bev[:16], conv3x3(cat(bev[16:], T), fw, fb))

Algebraic restructuring (validated exactly vs reference in fp32 numpy):
  * v only enters via vsum = v.sum(h): the 3x3 conv collapses to 1-D 3-tap
    convs over colsum(front_x) with row-0 / row-127 edge corrections.
  * k only needs colsum; q's 1x1 conv commutes with pooling - the 2x2
    pooling is folded into strided matmul rhs access patterns reading the
    bev halo bands.
  * conv3x3 over the x2-nearest-upsampled T decomposes into 4 output-parity
    phases, each a 2x2-tap conv on half-res Tp with parity-summed weights.
  * the dense bev-channel part of the fusion conv (Part A) is 9 shifted
    matmuls over zero-padded bf16 halo bands covering all 256 bev channels.

Schedule (v4): the PE is the bottleneck engine (~395 us of matmul
streaming); every other resource is scheduled so the PE never waits.
  * q runs FIRST (needs only the bev bands + Wq): its output is staged
    as q+qb in bf16 and multiplied by k-broadcast in one pass later,
    decoupling q from the front column-sum chain.
  * Part A for group 0 fills the PE while the front column sums land;
    vsum / k / e are injected between A chunks exactly when their inputs
    arrive; then the loop alternates A(g) / B(g).
  * Part A drains psum straight into the per-group output tile via
    ACT-engine copies; Part B scatter-adds its 4 phase outputs (+bias)
    into the same tile (DVE), which then DMAs out per mb block.
  * The front column-sum is chunked (8 rows per DMA) and reduced as
    chunks land: block 0 via DVE transposed tensor_reduce, block 1 via a
    contiguous halving tree on the Pool engine (no free-axis reduce
    there); e-column norms also run on the Pool engine.
  * DMA emission order = need order: bands+Wq+qb, WA (one batched
    descriptor, as are WB/WV), front chunks, small weights, WB, with the
    bev[:16] DRAM->DRAM passthrough queued last.
No all-engine barriers: no SBUF window is recycled across pools, and the
PSUM prefix->B pool handoff is ordered transitively through the Tp
dependency chain (B's matmuls wait on Tp, which sits behind every prefix
psum reader on the DVE/ACT/Pool queues).

A post-Tile `_dedup_ldweights` pass drops PE weight reloads for
consecutive same-stationary matmuls (q taps, e's replicated-ones), and
`_legalize_waits` splits multi-wait instructions (the TPB encoding has
one wait slot).
"""

import numpy as np
import ml_dtypes

import concourse.bass as bass
import concourse.mybir as mybir
from concourse.alu_op_type import AluOpType
from concourse.tile import TileContext
from concourse.bass_utils import run_bass_kernel_spmd

F32 = mybir.dt.float32
BF16 = mybir.dt.bfloat16
AX = mybir.AxisListType
AF = mybir.ActivationFunctionType

B, C, H, W = 8, 256, 128, 128
C8 = 32
CO = C - 16          # 240
HP = H // 2          # 64
WP = W // 2          # 64
NCORES = 8
HB = 130             # halo row length (128 + 2)
TPW = WP + 2         # 66
NB = 8               # output row groups of 16
MBLK = [(0, 128), (128, 112)]    # out-channel blocks of the 240
KBLK = [(0, 128), (128, 128)]    # input-channel blocks of 256
FCH = 16             # front colsum chunk rows


def _tap_groups(par):
    return [[0], [1, 2]] if par == 0 else [[0, 1], [2]]


def _tap_offsets(par):
    return [-1, 0] if par == 0 else [0, 1]


def _prep_inputs(inputs):
    bf = ml_dtypes.bfloat16
    qw = np.asarray(inputs["qw"], np.float32)[:, :, 0, 0]
    kw = np.asarray(inputs["kw"], np.float32)[:, :, 0, 0]
    vw = np.asarray(inputs["vw"], np.float32)
    vb = np.asarray(inputs["vb"], np.float32)
    qb = np.asarray(inputs["qb"], np.float32)
    kb = np.asarray(inputs["kb"], np.float32)
    fw = np.asarray(inputs["fw"], np.float32)
    fb = np.asarray(inputs["fb"], np.float32)

    W2 = vw.sum(axis=2)                               # [C, C, 3]
    WV = np.zeros((9, C, C), np.float32)              # [src*3+dx, cin, cout]
    for dx in range(3):
        WV[0 * 3 + dx] = W2[:, :, dx].T
        WV[1 * 3 + dx] = -vw[:, :, 0, dx].T           # -row127 correction
        WV[2 * 3 + dx] = -vw[:, :, 2, dx].T           # -row0 correction
    Wq = qw.T * 0.25                                  # [C, C8]
    Wk = kw.T / 256.0                                 # [C, C8]
    # Part A weights over all 256 bev channels, rows 0..15 zero.
    WA = np.zeros((9, C, CO), np.float32)             # [dy*3+dx, cin, o]
    fwA = np.transpose(fw[:, :CO], (2, 3, 1, 0))      # [dy, dx, cin240, o]
    WA[:, 16:, :] = fwA.reshape(9, CO, CO)
    fwB = fw[:, CO:]                                  # [240, 256, 3, 3]
    WB = np.zeros((16, C, CO), np.float32)            # [((ph*2+pw)*2+i)*2+j]
    for ph in range(2):
        for pw in range(2):
            for i, dys in enumerate(_tap_groups(ph)):
                for j, dxs in enumerate(_tap_groups(pw)):
                    acc = np.zeros((C, CO), np.float32)
                    for dy in dys:
                        for dx in dxs:
                            acc += fwB[:, :, dy, dx].T
                    WB[((ph * 2 + pw) * 2 + i) * 2 + j] = acc
    front = np.asarray(inputs["front_x"], np.float32)
    bev = np.asarray(inputs["bev_x"], np.float32)
    # big weights pre-flattened to their exact SBUF layout
    # [128p, (tap, kblk, cout)] so each loads as ONE contiguous DMA
    WAf = WA.reshape(9, 2, 128, CO).transpose(2, 0, 1, 3).reshape(128, -1)
    WBf = WB.reshape(16, 2, 128, CO).transpose(2, 0, 1, 3).reshape(128, -1)
    WVf = WV.reshape(9, 2, 128, C).transpose(2, 0, 1, 3).reshape(128, -1)
    shared = {
        "WV": np.ascontiguousarray(WVf).astype(bf),
        "Wq": Wq.astype(bf),
        "Wk": Wk.astype(bf),
        "WA": np.ascontiguousarray(WAf).astype(bf),
        "WB": np.ascontiguousarray(WBf).astype(bf),
        "vbias": (64.0 * vb).astype(np.float32),
        "qb": qb.astype(np.float32),
        "kb": kb.astype(np.float32),
        "fb": fb.astype(np.float32),
        "ones": np.ones((C8, 128), bf),
    }
    in_maps = []
    for b in range(NCORES):
        # bev pre-padded with zero columns in DRAM so every band DMA is
        # one fully contiguous burst per partition (the strided 256B
        # halo writes measured ~2x slower)
        bev_pad = np.zeros((C, H, HB), ml_dtypes.bfloat16)
        bev_pad[:, :, 1:1 + W] = bev[b].astype(bf)
        m = {
            "front_b": np.ascontiguousarray(front[b].astype(bf)),
            "bev_b": bev_pad,
            "bev16": np.ascontiguousarray(bev[b, :16]),
        }
        m.update(shared)
        in_maps.append(m)
    return in_maps


def _build_module():
    nc = bass.Bass()
    fx_d = nc.dram_tensor("front_b", [C, H, W], BF16, kind="ExternalInput")
    bx_d = nc.dram_tensor("bev_b", [C, H, HB], BF16, kind="ExternalInput")
    b16_d = nc.dram_tensor("bev16", [16, H, W], F32, kind="ExternalInput")
    WV_d = nc.dram_tensor("WV", [128, 18 * C], BF16, kind="ExternalInput")
    Wq_d = nc.dram_tensor("Wq", [C, C8], BF16, kind="ExternalInput")
    Wk_d = nc.dram_tensor("Wk", [C, C8], BF16, kind="ExternalInput")
    WA_d = nc.dram_tensor("WA", [128, 18 * CO], BF16, kind="ExternalInput")
    WB_d = nc.dram_tensor("WB", [128, 32 * CO], BF16, kind="ExternalInput")
    vbias_d = nc.dram_tensor("vbias", [C], F32, kind="ExternalInput")
    qb_d = nc.dram_tensor("qb", [C8], F32, kind="ExternalInput")
    kb_d = nc.dram_tensor("kb", [C8], F32, kind="ExternalInput")
    fb_d = nc.dram_tensor("fb", [CO], F32, kind="ExternalInput")
    ones_d = nc.dram_tensor("ones", [C8, 128], BF16, kind="ExternalInput")
    out_d = nc.dram_tensor("out", [C, H, W], F32, kind="ExternalOutput")

    with TileContext(nc) as tc:
        with (
            tc.tile_pool(name="weights", bufs=1) as wp,
            tc.tile_pool(name="bands", bufs=1) as bandp,
            tc.tile_pool(name="front", bufs=1) as frp,
            tc.tile_pool(name="pref", bufs=1) as prp,
            tc.tile_pool(name="stage", bufs=1) as stp,
            tc.tile_pool(name="gout", bufs=1) as gop,
            tc.tile_pool(name="psa", bufs=4, space="PSUM") as psa,
        ):
            # prefix PSUM pool: closed before Part B's pool opens (B's
            # matmuls are ordered after every prefix psum reader through
            # the Tp dependency chain)
            psp_cm = tc.tile_pool(name="psp", bufs=1, space="PSUM")
            psp = psp_cm.__enter__()
            # DMA queues: bands / WB / outputs ride the SP hardware queue
            # (nc.sync); weights / front chunks ride the Activation
            # hardware queue (nc.scalar) so the two streams land in
            # parallel.
            # ===== DMA wave 1: Wq+qb (ACT q), bands g0/g1 (SP q) ========
            bands = [[None, None] for _ in range(NB)]

            def load_band(g):
                for bl, (c0, cs) in enumerate(KBLK):
                    bt = bandp.tile([cs, 18 * HB], BF16,
                                    name=f"band_{g}_{bl}",
                                    tag=f"band_{g}_{bl}")
                    bands[g][bl] = bt
                    v = bt[:].rearrange("p (r c) -> p r c", r=18)
                    h_lo, r0, nrows = 16 * g - 1, 0, 18
                    if g == 0:
                        nc.gpsimd.memset(v[:, 0:1, :], 0.0)
                        h_lo, r0, nrows = 0, 1, 17
                    if g == NB - 1:
                        nc.gpsimd.memset(v[:, 17:18, :], 0.0)
                        nrows -= 1
                    nc.sync.dma_start(
                        out=v[:, r0:r0 + nrows, :],
                        in_=bx_d[c0:c0 + cs, h_lo:h_lo + nrows, :])

            Wq_t = []
            for kb_i, (k0, ks) in enumerate(KBLK):
                tq = wp.tile([ks, C8], BF16, name=f"Wq_{kb_i}",
                             tag=f"Wq_{kb_i}")
                nc.scalar.dma_start(out=tq[:], in_=Wq_d[k0:k0 + ks, :])
                Wq_t.append(tq)
            qb_t = wp.tile([C8, 1], F32, name="qb_t", tag="qb_t")
            nc.scalar.dma_start(out=qb_t[:], in_=qb_d[:].unsqueeze(1))
            load_band(0)

            qk_t = prp.tile([C8, HP * WP], BF16, name="qk_t", tag="qk_t")
            qbank = [None]

            def part_q(g):
                # (k is folded in later in one DVE pass, so q does not
                # wait on the front column sums.) Four 32-partition q
                # accumulators pack into one PSUM bank via PE tile
                # positions, so 8 groups are in flight and the ACT-engine
                # drains can lag without stalling the PE.
                if g % 2 == 0:
                    qbank[0] = psp.tile([128, 8 * WP], F32, name="psQ",
                                        tag="psQ", bufs=3)
                t = g % 2
                psq = qbank[0][64 * t:64 * t + C8, :]
                for kb_i in range(2):
                    bv = bands[g][kb_i][:].rearrange("p (r c) -> p r c",
                                                     c=HB)
                    for i in range(2):
                        for j in range(2):
                            rhs = bv[:, 1 + i:17 + i:2, 1 + j:129 + j:2]
                            nc.tensor.matmul(
                                psq, Wq_t[kb_i][:], rhs,
                                start=(kb_i == 0 and i == 0 and j == 0),
                                stop=(kb_i == 1 and i == 1 and j == 1))
                # drain on the DVE: q runs after A(1).mb0, when the DVE
                # is idle (trees done) but the ACT queue is congested
                # with Part-A psum copies
                nc.vector.tensor_scalar_add(
                    out=qk_t[:, g * 8 * WP:(g + 1) * 8 * WP],
                    in0=psq, scalar1=qb_t[:])

            # ===== DMA wave 2: WA batched + fb (ACT q) ==================
            WAbig = wp.tile([128, 18 * CO], BF16, name="WAbig", tag="WAbig")
            nc.scalar.dma_start(out=WAbig[:], in_=WA_d[:])

            def wa_ap(sd, kb_i, m0, ms):
                off = (sd * 2 + kb_i) * CO
                return WAbig[:, off + m0:off + m0 + ms]

            fb_t = []
            for mb_i, (m0, ms) in enumerate(MBLK):
                t = wp.tile([ms, 1], F32, name=f"fb_{mb_i}", tag=f"fb_{mb_i}")
                nc.scalar.dma_start(out=t[:],
                                    in_=fb_d[m0:m0 + ms].unsqueeze(1))
                fb_t.append(t)

            # ================= Part A emitter (per group) ===============
            ot_tiles = {}

            def part_a(g, mb_list=(0, 1)):
                """Fused-conv Part A for row group g: psum accumulate then
                ACT-engine copy into the group output tile."""
                for mb_i in mb_list:
                    m0, ms = MBLK[mb_i]
                    par = g % 3 if mb_i == 0 else g % 2
                    ot = gop.tile([ms, 16 * W], F32,
                                  name=f"ot{par}_{mb_i}",
                                  tag=f"ot{par}_{mb_i}")
                    ot_tiles[(g, mb_i)] = ot
                    for n in range(4):
                        pa_ = psa.tile([ms, 4 * W], F32, name="psAt",
                                       tag="psAt")
                        first = True
                        for dy in range(3):
                            for dx in range(3):
                                for kb_i in range(2):
                                    bv = bands[g][kb_i][:].rearrange(
                                        "p (r c) -> p r c", c=HB)
                                    rhs = bv[:, 4 * n + dy:4 * n + dy + 4,
                                             dx:dx + W]
                                    nc.tensor.matmul(
                                        pa_[:],
                                        wa_ap(dy * 3 + dx, kb_i, m0, ms),
                                        rhs,
                                        start=first,
                                        stop=(dy == 2 and dx == 2
                                              and kb_i == 1))
                                    first = False
                        nc.scalar.copy(
                            out=ot[:, n * 4 * W:(n + 1) * 4 * W],
                            in_=pa_[:])

            # ===== PE wave 1: A(0), A(1).mb0 under the input DMAs =======
            # (only needs WA + the first bands; fills ~80us of PE while
            # the remaining bands / front / weights land)
            load_band(1)
            part_a(0)
            part_a(1, mb_list=(0,))

            # ===== DMA wave 3: bands g1..7 + WB (SP q); front chunks
            # (ACT q) with the colsum trees chasing them; the small
            # weights follow (needed only by vsum/e, much later) =========
            for g in range(2, NB):
                load_band(g)
            WBbig = wp.tile([128, 32 * CO], BF16, name="WBbig", tag="WBbig")
            nc.sync.dma_start(out=WBbig[:], in_=WB_d[:])

            def wb_ap(cc, kb_i, m0, ms):
                off = (cc * 2 + kb_i) * CO
                return WBbig[:, off + m0:off + m0 + ms]

            # colsum: per-chunk contiguous halving trees, chunks
            # round-robined across the DVE and the Pool engine (each keeps
            # its own partial sum; combined later at x3 assembly)
            csum_t, X3b, P2b = [], [], []
            NCH = H // FCH
            engs = [nc.vector, nc.gpsimd]
            csum_p = [[None, None], [None, None]]   # [bl][eng]
            for bl in range(2):
                c0 = bl * 128
                for chunk in range(NCH):
                    e_i = chunk % 2
                    eng = engs[e_i]
                    en = "v" if e_i == 0 else "g"
                    ch = frp.tile([128, FCH * W], BF16,
                                  name=f"fch_{bl}_{chunk}",
                                  tag=f"fch_{en}", bufs=1)
                    nc.scalar.dma_start(
                        out=ch[:],
                        in_=fx_d[c0:c0 + 128,
                                 chunk * FCH:(chunk + 1) * FCH, :])
                    # in-place halving tree inside the chunk tile
                    sz = FCH * W // 2
                    while sz > W:
                        eng.tensor_tensor(
                            out=ch[:, 0:sz], in0=ch[:, 0:sz],
                            in1=ch[:, sz:2 * sz], op=AluOpType.add)
                        sz //= 2
                    lvl = frp.tile([128, W], F32, name=f"trf_{en}",
                                   tag=f"trf_{en}")
                    eng.tensor_tensor(out=lvl[:], in0=ch[:, 0:W],
                                      in1=ch[:, W:2 * W], op=AluOpType.add)
                    if chunk < 2:
                        cs = frp.tile([128, W], F32,
                                      name=f"colsum_{bl}_{en}",
                                      tag=f"colsum_{bl}_{en}")
                        csum_p[bl][e_i] = cs
                        eng.tensor_copy(out=cs[:], in_=lvl[:])
                    else:
                        cs = csum_p[bl][e_i]
                        eng.tensor_add(out=cs[:], in0=cs[:], in1=lvl[:])

            Wk_t = []
            for kb_i, (k0, ks) in enumerate(KBLK):
                tk = wp.tile([ks, C8], BF16, name=f"Wk_{kb_i}",
                             tag=f"Wk_{kb_i}")
                nc.scalar.dma_start(out=tk[:], in_=Wk_d[k0:k0 + ks, :])
                Wk_t.append(tk)
            WVbig = wp.tile([128, 18 * C], BF16, name="WVbig", tag="WVbig")
            nc.scalar.dma_start(out=WVbig[:], in_=WV_d[:])

            def wv_ap(sd, kb_i, mb):
                off = (sd * 2 + kb_i) * C
                return WVbig[:, off + mb * 128:off + (mb + 1) * 128]

            ones_t = wp.tile([C8, 128], BF16, name="ones_t", tag="ones_t")
            nc.scalar.dma_start(out=ones_t[:], in_=ones_d[:])
            vbias_t = []
            for bl in range(2):
                t = wp.tile([128, 1], F32, name=f"vbias_{bl}",
                            tag=f"vbias_{bl}")
                nc.scalar.dma_start(
                    out=t[:], in_=vbias_d[bl * 128:(bl + 1) * 128].unsqueeze(1))
                vbias_t.append(t)
            kb_t = wp.tile([C8, 1], F32, name="kb_t", tag="kb_t")
            nc.scalar.dma_start(out=kb_t[:], in_=kb_d[:].unsqueeze(1))

            # ===== PE wave 2: q(g0..g7) (all bands landed by now) =======
            for g in range(NB):
                part_q(g)
            part_a(1, mb_list=(1,))

            # ===== x3 / p2 assembly (DVE; inputs from both engines) =====
            for bl in range(2):
                c0 = bl * 128
                csum = csum_p[bl][0]
                csum_t.append(csum)
                nc.vector.tensor_add(out=csum[:], in0=csum[:],
                                     in1=csum_p[bl][1][:])
                r0t = frp.tile([128, W], BF16, name=f"r0_{bl}",
                               tag=f"r0_{bl}")
                rLt = frp.tile([128, W], BF16, name=f"rL_{bl}",
                               tag=f"rL_{bl}")
                nc.scalar.dma_start(out=r0t[:], in_=fx_d[c0:c0 + 128, 0, :])
                nc.scalar.dma_start(out=rLt[:],
                                    in_=fx_d[c0:c0 + 128, H - 1, :])
                x3v = frp.tile([128, 3 * HB], BF16, name=f"x3_{bl}",
                               tag=f"x3_{bl}")
                xv = x3v[:].rearrange("p (s c) -> p s c", s=3)
                nc.gpsimd.memset(xv[:, :, 0:1], 0.0)
                nc.gpsimd.memset(xv[:, :, HB - 1:HB], 0.0)
                nc.vector.tensor_copy(out=xv[:, 0, 1:1 + W], in_=csum[:])
                nc.vector.tensor_copy(out=xv[:, 1, 1:1 + W], in_=rLt[:])
                nc.vector.tensor_copy(out=xv[:, 2, 1:1 + W], in_=r0t[:])
                X3b.append(xv)
                p2 = frp.tile([128, WP], BF16, name=f"p2_{bl}",
                              tag=f"p2_{bl}")
                cs3 = csum[:].rearrange("p (w two) -> p w two", two=2)
                nc.vector.tensor_tensor(out=p2[:], in0=cs3[:, :, 0],
                                        in1=cs3[:, :, 1], op=AluOpType.add)
                P2b.append(p2)

            # ================= Part B emitter (per group) ===============
            tp_t = []

            def part_b(psb, g):
                """Part B phases for group g: scatter-add (+bias) into the
                group output tile, then DMA the tile out."""
                for mb_i, (m0, ms) in enumerate(MBLK):
                    ot = ot_tiles.pop((g, mb_i))
                    sv = ot[:].rearrange(
                        "p (h two w pw2) -> p h two w pw2",
                        two=2, w=WP, pw2=2)
                    for ph in range(2):
                        ro = _tap_offsets(ph)
                        for pw in range(2):
                            co = _tap_offsets(pw)
                            pb_ = psb.tile([ms, 8 * WP], F32, name="psBt",
                                           tag="psBt")
                            first = True
                            for i in range(2):
                                for j in range(2):
                                    cc = ((ph * 2 + pw) * 2 + i) * 2 + j
                                    for kb_i in range(2):
                                        tv = tp_t[kb_i][:].rearrange(
                                            "p (r c) -> p r c", c=TPW)
                                        rhs = tv[:,
                                                 8 * g + 1 + ro[i]:
                                                 8 * g + 9 + ro[i],
                                                 1 + co[j]:
                                                 1 + co[j] + WP]
                                        nc.tensor.matmul(
                                            pb_[:],
                                            wb_ap(cc, kb_i, m0, ms),
                                            rhs,
                                            start=first,
                                            stop=(i == 1 and j == 1
                                                  and kb_i == 1))
                                        first = False
                            osl = sv[:, :, ph, :, pw]
                            nc.vector.scalar_tensor_tensor(
                                out=osl,
                                in0=pb_[:].rearrange("p (h w) -> p h w",
                                                     w=WP),
                                scalar=fb_t[mb_i][:], in1=osl,
                                op0=AluOpType.add, op1=AluOpType.add)
                    nc.sync.dma_start(
                        out=out_d[16 + m0:16 + m0 + ms,
                                  16 * g:16 * (g + 1), :],
                        in_=ot[:].rearrange("p (r c) -> p r c", c=W))

            # ---- vsum (PE; column sums just landed) ----
            vsum_t = []
            for mb in range(2):
                ps = psp.tile([128, W], F32, name="psS", tag="psS")
                first = True
                for sd in range(9):
                    src, dx = divmod(sd, 3)
                    for kb_i in range(2):
                        nc.tensor.matmul(
                            ps[:], wv_ap(sd, kb_i, mb),
                            X3b[kb_i][:, src, dx:dx + W],
                            start=first, stop=(sd == 8 and kb_i == 1))
                        first = False
                ssb = prp.tile([128, W], F32, name=f"ssb_{mb}",
                               tag=f"ssb_{mb}")
                nc.scalar.activation(out=ssb[:], in_=ps[:],
                                     func=AF.Copy, scale=0.25)
                se = ssb[:].rearrange("p (w two) -> p w two", two=2)
                vs = prp.tile([128, WP], F32, name=f"vsum_{mb}",
                              tag=f"vsum_{mb}")
                nc.vector.scalar_tensor_tensor(
                    out=vs[:], in0=se[:, :, 0],
                    scalar=vbias_t[mb][:], in1=se[:, :, 1],
                    op0=AluOpType.add, op1=AluOpType.add)
                vsum_t.append(vs)

            # ---- k, then fold k into the staged q (one DVE pass) ----
            psk = psp.tile([C8, WP], F32, name="psK", tag="psS")
            nc.tensor.matmul(psk[:], Wk_t[0][:], P2b[0][:],
                             start=True, stop=False)
            nc.tensor.matmul(psk[:], Wk_t[1][:], P2b[1][:],
                             start=False, stop=True)
            k_t = prp.tile([C8, WP], F32, name="k_t", tag="k_t")
            nc.vector.tensor_scalar_add(out=k_t[:], in0=psk[:],
                                        scalar1=kb_t[:])
            qkv = qk_t[:].rearrange("p (h w) -> p h w", w=WP)
            kv = k_t[:].unsqueeze(1).broadcast_to([C8, HP, WP])
            nc.vector.tensor_tensor(out=qkv, in0=qkv, in1=kv,
                                    op=AluOpType.mult)

            # ---- e (replicated) + column norms (Pool engine) ----
            e_t = prp.tile([128, HP * WP], BF16, name="e_t", tag="e_t")
            n2 = prp.tile([128, WP], F32, name="n2", tag="n2")
            for chn in range(8):
                nsl = slice(chn * 512, (chn + 1) * 512)
                pse = psp.tile([128, 512], F32, name="psE", tag="psQ",
                               bufs=3)
                nc.tensor.matmul(pse[:], ones_t[:], qk_t[:, nsl],
                                 start=True, stop=True)
                nc.vector.tensor_copy(out=e_t[:, nsl], in_=pse[:])
                scr = prp.tile([128, 8 * WP], F32, name="scr",
                               tag="scr", bufs=2)
                esl = e_t[:, chn * 8 * WP:(chn + 1) * 8 * WP]
                nc.gpsimd.tensor_tensor(out=scr[:], in0=esl, in1=esl,
                                        op=AluOpType.mult)
                lvl, sz = scr, 4 * WP
                for d in range(3):
                    nxt = prp.tile([128, sz], F32, name=f"nt{d}",
                                   tag=f"nt{d}", bufs=2)
                    nc.gpsimd.tensor_tensor(
                        out=nxt[:], in0=lvl[:, 0:sz],
                        in1=lvl[:, sz:2 * sz], op=AluOpType.add)
                    lvl, sz = nxt, sz // 2
                if chn == 0:
                    nc.gpsimd.tensor_copy(out=n2[:], in_=lvl[:])
                else:
                    nc.gpsimd.tensor_add(out=n2[:], in0=n2[:], in1=lvl[:])
            nrm = prp.tile([128, WP], F32, name="nrm", tag="nrm")
            nc.scalar.sqrt(out=nrm[:], in_=n2[:])
            rinv = prp.tile([128, WP], F32, name="rinv", tag="rinv")
            nc.vector.reciprocal(out=rinv[:], in_=nrm[:])

            # ---- Tp_pad = (vsum * rinv) x e ----
            for bl in range(2):
                teng = nc.vector if bl == 0 else nc.gpsimd
                vs2 = prp.tile([128, WP], F32, name=f"vs2_{bl}",
                               tag=f"vs2_{bl}")
                teng.tensor_tensor(
                    out=vs2[:], in0=vsum_t[bl][:], in1=rinv[:],
                    op=AluOpType.mult)
                tp = stp.tile([128, (HP + 2) * TPW], BF16,
                              name=f"tp_{bl}", tag=f"tp_{bl}")
                tp_t.append(tp)
                tv = tp[:].rearrange("p (r c) -> p r c", c=TPW)
                nc.gpsimd.memset(tv[:, 0:1, :], 0.0)
                nc.gpsimd.memset(tv[:, HP + 1:HP + 2, :], 0.0)
                nc.gpsimd.memset(tv[:, :, 0:1], 0.0)
                nc.gpsimd.memset(tv[:, :, TPW - 1:TPW], 0.0)
                ev = e_t[:].rearrange("p (h w) -> p h w", w=WP)
                v2 = vs2[:].unsqueeze(1).broadcast_to([128, HP, WP])
                teng.tensor_tensor(
                    out=tv[:, 1:1 + HP, 1:1 + WP], in0=v2, in1=ev,
                    op=AluOpType.mult)

            # ===== PE wave 3: B(g) sandwiched between A(g+2) halves so
            # Part B's Tp/psum deps never stall the PE ===================
            psp_cm.__exit__(None, None, None)
            with tc.tile_pool(name="psb", bufs=3, space="PSUM") as psb:
                for g in range(2, NB):
                    part_a(g, mb_list=(0,))
                    part_b(psb, g - 2)
                    part_a(g, mb_list=(1,))
                part_b(psb, NB - 2)
                part_b(psb, NB - 1)

            # out[:16] = bev[:16] straight through, DRAM->DRAM (queued
            # last: no consumer inside the kernel)
            nc.sync.dma_start(out=out_d[0:16], in_=b16_d[:])
    return nc


def _dedup_ldweights(nc):
    """Tile splits every matmul into a standalone InstLdweights plus a
    non-self-loading InstMatmult. Consecutive matmuls reusing the same
    stationary therefore emit redundant PE-array loads. Delete an
    InstLdweights when the previous one on the PE stream loaded the
    identical weights AP and nothing in between clobbered the array.
    Ldweights carry no semaphore updates; any vestigial waits are folded
    into the next matmul (split later by `_legalize_waits` if needed)."""
    n_drop = 0
    PE = mybir.EngineType.PE
    for fn in nc.m.functions:
        for bb in fn.blocks:
            out = []
            last_key = None
            pending_waits = []
            for ins in bb.instructions:
                if isinstance(ins, mybir.InstLdweights):
                    key = (str(ins.ins[0]), str(ins.tile_position),
                           str(ins.tile_size), str(ins.perf_mode),
                           str(ins.is_transpose))
                    if key == last_key:
                        si = ins.sync_info
                        if si is not None and si.on_update:
                            out.append(ins)   # cannot drop: has updates
                            continue
                        if si is not None and si.on_wait:
                            pending_waits.extend(si.on_wait)
                        n_drop += 1
                        continue
                    last_key = key
                    out.append(ins)
                elif isinstance(ins, mybir.InstMatmult):
                    if ins.is_transpose:
                        last_key = None
                    if pending_waits:
                        si = ins.sync_info
                        w = list(si.on_wait) if si else []
                        u = list(si.on_update) if si else []
                        ins.sync_info = mybir.SyncInfo(
                            on_wait=w + pending_waits, on_update=u)
                        pending_waits = []
                    out.append(ins)
                else:
                    if (getattr(ins, "engine", None) == PE
                            and not isinstance(ins,
                                               mybir.InstEventSemaphore)):
                        last_key = None
                    out.append(ins)
            assert not pending_waits, "dangling ldweights waits after dedup"
            bb.instructions[:] = out
    return n_drop


def _legalize_waits(nc):
    """This toolchain's codegen accepts at most ONE semaphore wait per
    instruction (the TPB `events` field has a single wait slot). Tile's
    wait assignment can attach several. Hoist all but one wait onto
    standalone EventSemaphore instructions placed immediately before the
    owner on the same engine stream - strictly stronger synchronization,
    so always safe."""
    n_split = 0
    for fn in nc.m.functions:
        for bb in fn.blocks:
            out = []
            for ins in bb.instructions:
                si = ins.sync_info
                if si is not None and len(si.on_wait) > 1:
                    extra = list(si.on_wait[:-1])
                    keep = si.on_wait[-1]
                    for idx, wt in enumerate(extra):
                        ev = mybir.InstEventSemaphore(
                            name=f"{ins.name}_hw{idx}",
                            engine=ins.engine,
                            sync_info=mybir.SyncInfo(on_wait=[wt],
                                                     on_update=[]),
                        )
                        out.append(ev)
                        n_split += 1
                    ins.sync_info = mybir.SyncInfo(
                        on_wait=[keep], on_update=list(si.on_update))
                out.append(ins)
            bb.instructions[:] = out
    return n_split


_NC_CACHE = None


def kernel(**inputs):
    global _NC_CACHE
    in_maps = _prep_inputs(inputs)
    if _NC_CACHE is None:
        _NC_CACHE = _build_module()
        _dedup_ldweights(_NC_CACHE)
        _legalize_waits(_NC_CACHE)
    res = run_bass_kernel_spmd(_NC_CACHE, in_maps, list(range(NCORES)))
    out = np.stack([res.results[b]["out"] for b in range(NCORES)], axis=0)
    return out.astype(np.float32)
